# revision 9
# baseline (speedup 1.0000x reference)
"""Causal self-attention (T=2048, C=1024, H=16) on 8 trn2 NeuronCores.

Tensor-parallel over heads: core i computes heads 2i, 2i+1 (q/k/v rows
128i:128i+128 of each 1024-row block of wqkv_w, proj_w columns
128i:128i+128), producing a partial output projection; partials are summed
on the host (the all-reduce of the sharding hint).

Per-core Bass/Tile kernel, bf16 matmuls with fp32 PSUM accumulation.
Key structure (v2):
  B. qkvT[j, t] = wqkv.T @ xT, contraction-tile outer so matmuls chase the
     x DMAs (x tiles spread over 3 DMA queues); q/k psum held as [128,2,CH]
     2-bank tiles so each evacuation is one [128,1024] DVE op. v's PE
     transposes are full 128x128 blocks (both heads at once) feeding
     v_aug[k, 65] (ones column = softmax denominator via the PV matmul).
  D. per 512-col t-chunk, k-tiles in PAIRS: both scores of a pair land in
     one [128,2,CH] 2-bank PSUM tile, one [128,<=1024] exp ACTIVATE per
     pair per head (halves ScalarE instruction overhead; the two heads'
     score matmuls row-tile concurrently on the PE via base_partition 0/64).
     Causal affine_select on gpsimd (diagonal k-tiles only); PV pipelined
     behind the scores; the previous chunk's normalize/proj matmuls spread
     through the pair-loop as PE filler.
     Normalize: 1/sums on the DVE (vector.reciprocal on the [1,1024] sums
     row - no ScalarE Ln/Exp, no act-table switches), partition-broadcast
     via K=1 float32r matmul with ones, one DVE multiply per head writing
     into the shared attn[128, T] tile (head h at partitions 64h:64h+64).
  E. partialT[o, t] = projT.T @ attn: ONE K=128 matmul per 128-col o-tile
     (both heads contracted together), evacuated bf16 and stored as
     contiguous 128KB DMAs on rotating queues.
"""

import sys

if "/opt/trn_rl_repo" not in sys.path:
    sys.path.insert(0, "/opt/trn_rl_repo")

import ml_dtypes
import numpy as np

T = 2048
C = 1024
CH = 512  # t-chunk width (one PSUM bank of fp32)
NT = T // CH  # 4 t-chunks
NK = T // 128  # 16 k-tiles
NCT = C // 128  # 8 contraction tiles
N_CORES = 8
PIPE = 4  # scores->PV pipeline depth in k-tile steps

_CACHE = {}


def _build():
    import concourse.tile as tile
    from concourse import bacc, mybir

    F32 = mybir.dt.float32
    F32R = mybir.dt.float32r
    BF16 = mybir.dt.bfloat16
    EXP = mybir.ActivationFunctionType.Exp
    IS_GE = mybir.AluOpType.is_ge

    nc = bacc.Bacc(
        "TRN2",
        target_bir_lowering=False,
        debug=False,
        enable_asserts=False,
        num_devices=N_CORES,
        num_swdge_queues=4,
    )
    xT = nc.dram_tensor("xT", [C, T], BF16, kind="ExternalInput").ap()
    wqkv = nc.dram_tensor("wqkv", [C, 384], BF16, kind="ExternalInput").ap()
    projT = nc.dram_tensor("projT", [128, C], BF16, kind="ExternalInput").ap()
    identb = nc.dram_tensor("identb", [128, 128], BF16, kind="ExternalInput").ap()
    ones_f = nc.dram_tensor("ones_f", [128, 128], F32R, kind="ExternalInput").ap()
    bias = nc.dram_tensor("bias", [128, 3], F32, kind="ExternalInput").ap()
    # output as contiguous [chunk, o-tile, 128, 512] bf16 tiles: each store is
    # one fully-contiguous 128KB DMA
    out = nc.dram_tensor("out", [NT, 8, 128, CH], BF16, kind="ExternalOutput").ap()

    with tile.TileContext(nc) as tc:
        with (
            tc.tile_pool(name="big", bufs=1) as big,
            tc.tile_pool(name="expw", bufs=6) as expw_pool,
            tc.tile_pool(name="attn_tmp", bufs=2) as attn_tmp_pool,
            tc.tile_pool(name="outev", bufs=3) as outev_pool,
            tc.tile_pool(name="ps", bufs=1, space="PSUM") as ps,
        ):
            # ---- resident SBUF tensors -------------------------------------
            x_sb = big.tile([128, NCT, T], BF16, name="x_sb")
            w_sb = big.tile([128, NCT, 384], BF16, name="w_sb")
            projT_sb = big.tile([128, C], BF16, name="projT_sb")
            qT_sb = big.tile([128, T], BF16, name="qT_sb")
            kT_sb = big.tile([128, T], BF16, name="kT_sb")
            vT_sb = big.tile([128, T], BF16, name="vT_sb")
            v_aug0 = big.tile([128, NK, 65], BF16, name="v_aug0")
            v_aug1 = big.tile([128, NK, 65], BF16, name="v_aug1")
            attn_sb = big.tile([128, T], BF16, name="attn_sb")
            ident_sb = big.tile([128, 128], BF16, name="ident_sb")
            ones_sb = big.tile([128, 128], F32R, name="ones_sb")
            bias_sb = big.tile([128, 3], F32, name="bias_sb")

            # x and w tiles are fully-contiguous DRAM regions (full rows), so
            # each DMA streams at queue peak; x tiles spread across 3 queues
            # (scalar HWDGE, gpsimd SWDGE, sync HWDGE - all idle this early)
            # so the 4MB load isn't bottlenecked on one queue's rate.
            xengs = [nc.scalar] * 3 + [nc.gpsimd] * 3 + [nc.sync] * 2
            for ct in range(NCT):
                nc.sync.dma_start(
                    out=w_sb[:, ct, :], in_=wqkv[128 * ct : 128 * ct + 128, :]
                )
                xengs[ct].dma_start(
                    out=x_sb[:, ct, :], in_=xT[128 * ct : 128 * ct + 128, :]
                )
            nc.sync.dma_start(out=bias_sb, in_=bias)
            nc.sync.dma_start(out=ident_sb, in_=identb)
            nc.sync.dma_start(out=ones_sb, in_=ones_f)
            nc.sync.dma_start(out=projT_sb, in_=projT)

            nc.vector.memset(v_aug0[:, :, 64:65], 1.0)
            nc.vector.memset(v_aug1[:, :, 64:65], 1.0)

            # ---- stage B: q/k projections, two ct-outer sweeps -------------
            # part 0 = q, part 1 = k; one [128, 2, CH] psum tile per part
            # holds the two 512-col t-chunks of this sweep.
            for sweep in range(2):
                grp = {}
                for part in (0, 1):
                    grp[part] = ps.tile(
                        [128, 2, CH], F32, tag="s2", bufs=2, name=f"qkps_{sweep}_{part}"
                    )
                for ct in range(NCT):
                    for part, g in grp.items():
                        cols = slice(128 * part, 128 * part + 128)
                        for half in (0, 1):
                            c = 2 * sweep + half
                            nc.tensor.matmul(
                                g[:, half, :],
                                w_sb[:, ct, cols],
                                x_sb[:, ct, CH * c : CH * c + CH],
                                start=(ct == 0),
                                stop=(ct == NCT - 1),
                            )
                for part, g in grp.items():
                    dest = qT_sb if part == 0 else kT_sb
                    nc.vector.tensor_scalar_add(
                        dest[:, CH * 2 * sweep : CH * 2 * sweep + 2 * CH],
                        g,
                        bias_sb[:, part : part + 1],
                    )

            # v projection + PE transposes, per chunk; chunks 0,1 up front,
            # the rest emitted as PE filler inside stage D's pair-loops.
            def emit_v_chunk(c):
                v_ps = ps.tile([128, CH], F32, tag="m", bufs=2, name=f"vps_{c}")
                for ct in range(NCT):
                    nc.tensor.matmul(
                        v_ps,
                        w_sb[:, ct, 256:384],
                        x_sb[:, ct, CH * c : CH * c + CH],
                        start=(ct == 0),
                        stop=(ct == NCT - 1),
                    )
                nc.vector.tensor_scalar_add(
                    vT_sb[:, CH * c : CH * c + CH], v_ps, bias_sb[:, 2:3]
                )

            def transposes_for(c):
                # full 128x128 transposes: both heads' v in one shot
                for kt in range(4 * c, 4 * c + 4):
                    tr_ps = ps.tile([128, 128], BF16, tag="m", bufs=2, name=f"tr_{kt}")
                    nc.tensor.transpose(
                        tr_ps, vT_sb[:, 128 * kt : 128 * kt + 128], ident_sb
                    )
                    nc.vector.tensor_copy(v_aug0[:, kt, 0:64], tr_ps[:, 0:64])
                    nc.vector.tensor_copy(v_aug1[:, kt, 0:64], tr_ps[:, 64:128])

            emit_v_chunk(0)
            transposes_for(0)
            emit_v_chunk(1)
            transposes_for(1)

            # ---- stages D+E per t-chunk ------------------------------------
            # Deferred work from chunk c-1, spread through chunk c's pair-loop.
            pending_norm = None  # (at2, chunk)
            pending_proj = None  # chunk index

            def emit_norm(at2, pc):
                tcol = slice(CH * pc, CH * pc + CH)
                for h in (0, 1):
                    rb_ps = ps.tile([64, CH], F32, tag="m", bufs=2, name=f"rb_{h}_{pc}")
                    nc.tensor.matmul(
                        rb_ps,
                        ones_sb[64:65, 0:64],
                        at2[64:65, h, :],
                        start=True,
                        stop=True,
                    )
                    nc.vector.tensor_mul(
                        attn_sb[64 * h : 64 * h + 64, tcol], at2[0:64, h, :], rb_ps
                    )

            store_engs = [nc.sync, nc.sync, nc.sync, nc.gpsimd]

            def emit_proj_tile(pc, m):
                tcol = slice(CH * pc, CH * pc + CH)
                pr_ps = ps.tile([128, CH], F32, tag="m", bufs=2, name=f"pr_{m}_{pc}")
                nc.tensor.matmul(
                    pr_ps,
                    projT_sb[:, 128 * m : 128 * m + 128],
                    attn_sb[:, tcol],
                    start=True,
                    stop=True,
                )
                ob = outev_pool.tile([128, CH], BF16, tag="outev", name=f"ob_{m}_{pc}")
                nc.vector.tensor_copy(ob, pr_ps)
                store_engs[m % 4].dma_start(out=out[pc, m], in_=ob)

            # Chunk order (1, 2, 3, 0): the ScalarE-heavy late chunks get the
            # previous chunk's projection matmuls as PE filler, and the final
            # chunk processed (0) has the shortest tail.
            for c in (1, 2, 3, 0):
                npair = 2 * c + 2
                pv_ps = {
                    h: ps.tile([65, CH], F32, tag="pv", bufs=2, name=f"pv_{h}_{c}")
                    for h in (0, 1)
                }
                pending = []
                proj_emitted = 0

                def emit_pv(step):
                    for (pj, ph, pw, plane, plo) in step:
                        nc.tensor.matmul(
                            pv_ps[ph][:, plo:CH],
                            (v_aug0 if ph == 0 else v_aug1)[:, pj, :],
                            pw[:, plane, plo:CH],
                            start=(pj == 0),
                            stop=(pj == 4 * c + 3),
                        )

                for P in range(npair):
                    for h in (0, 1):
                        hrow = slice(64 * h, 64 * h + 64)
                        s2 = ps.tile(
                            [128, 2, CH], F32, tag="s2", bufs=2, name=f"s_{h}_{c}_{P}"
                        )
                        # pair-level column offset: both planes computed from
                        # slo_pair so the pair exp reads only written PSUM
                        # (the second diagonal plane recomputes 128 masked
                        # cols - 128 cycles, cheaper than a separate ACT)
                        slo_pair = max(0, 128 * (2 * P - 4 * c))
                        for i in (0, 1):
                            j = 2 * P + i
                            nc.tensor.matmul(
                                s2[:, i, slo_pair:CH],
                                kT_sb[hrow, 128 * j : 128 * j + 128],
                                qT_sb[hrow, CH * c + slo_pair : CH * c + CH],
                                start=True,
                                stop=True,
                            )
                        w2 = expw_pool.tile(
                            [128, 2, CH], BF16, tag="expw", name=f"w_{h}_{c}_{P}"
                        )
                        nc.scalar.activation(
                            out=w2[:, :, slo_pair:CH],
                            in_=s2[:, :, slo_pair:CH],
                            func=EXP,
                        )
                        for i in (0, 1):
                            j = 2 * P + i
                            diag = j - 4 * c
                            if diag >= 0:
                                # keep exp(score) where t >= k: within the
                                # kept column range f' = f - 128*diag, so
                                # f' - p >= 0
                                lo = 128 * diag if diag > 0 else 0
                                nc.gpsimd.affine_select(
                                    out=w2[:, i, lo:CH],
                                    in_=w2[:, i, lo:CH],
                                    pattern=[[1, CH - lo]],
                                    compare_op=IS_GE,
                                    fill=0.0,
                                    base=0,
                                    channel_multiplier=-1,
                                )
                            pending.append((j, h, w2, i, max(0, 128 * diag)))
                    if P == 0 and pending_norm is not None:
                        emit_norm(*pending_norm)
                        pending_norm = None
                    if P == 1 and c == 1:
                        emit_v_chunk(2)
                    if P == 2 and c == 1:
                        transposes_for(2)
                    if P == 1 and c == 3:
                        emit_v_chunk(3)
                    if P == 3 and c == 3:
                        transposes_for(3)
                    while len(pending) > 2 * PIPE:
                        step, pending = pending[:2], pending[2:]
                        emit_pv(step)
                    if pending_proj is not None and P >= 1:
                        target = (P * 8) // max(npair - 1, 1)
                        while proj_emitted < min(target, 8):
                            emit_proj_tile(pending_proj, proj_emitted)
                            proj_emitted += 1
                while pending:
                    step, pending = pending[:2], pending[2:]
                    emit_pv(step)
                if pending_proj is not None:
                    while proj_emitted < 8:
                        emit_proj_tile(pending_proj, proj_emitted)
                        proj_emitted += 1

                at2 = attn_tmp_pool.tile(
                    [65, 2, CH], F32R, tag="attn_tmp", name=f"at2_{c}"
                )
                for h in (0, 1):
                    nc.vector.tensor_copy(at2[:, h, :], pv_ps[h])
                # 1/sums on the DVE (row 64 holds the denominators); f32r out
                # so the broadcast matmul sees a properly-rounded f32r input
                rrow = at2[64:65, :, :]
                with nc.allow_low_precision(reason="1/den feeds f32r matmul"):
                    nc.vector.reciprocal(rrow, rrow)
                pending_norm = (at2, c)
                pending_proj = c

            emit_norm(*pending_norm)
            for m in range(8):
                emit_proj_tile(pending_proj, m)

    nc.compile()
    return nc


def _get_nc():
    if "nc" not in _CACHE:
        _CACHE["nc"] = _build()
    return _CACHE["nc"]


def _make_in_maps(x, wqkv_w, wqkv_b, proj_w):
    bf = ml_dtypes.bfloat16
    xT = np.ascontiguousarray(np.asarray(x, np.float32).T.astype(bf))
    identb = np.eye(128, dtype=bf)
    ones_f = np.ones((128, 128), np.float32)
    scale = np.float32(1.0 / np.sqrt(C))
    in_maps = []
    for i in range(N_CORES):
        rows = []
        biases = []
        for blk, s in ((0, scale), (1, None), (2, None)):
            sl = slice(blk * C + 128 * i, blk * C + 128 * i + 128)
            w = np.asarray(wqkv_w[sl], np.float32)
            b = np.asarray(wqkv_b[sl], np.float32)
            if s is not None:
                w = w * s
                b = b * s
            rows.append(w)
            biases.append(b)
        W = np.concatenate(rows, axis=0)  # [384, 1024]
        B = np.stack(biases, axis=1)  # [128, 3]
        pT = np.asarray(proj_w[:, 128 * i : 128 * i + 128], np.float32).T  # [128, 1024]
        in_maps.append(
            {
                "xT": xT,
                "wqkv": np.ascontiguousarray(W.T.astype(bf)),
                "projT": np.ascontiguousarray(pT.astype(bf)),
                "identb": identb,
                "ones_f": ones_f,
                "bias": np.ascontiguousarray(B),
            }
        )
    return in_maps


def kernel(x, wqkv_w, wqkv_b, proj_w, proj_b, _trace=False, _tmpdir=None):
    from concourse.bass_utils import run_bass_kernel_spmd

    nc = _get_nc()
    in_maps = _make_in_maps(x, wqkv_w, wqkv_b, proj_w)
    res = run_bass_kernel_spmd(
        nc,
        in_maps,
        core_ids=list(range(N_CORES)),
        trace=_trace,
        tmpdir=_tmpdir,
    )
    acc = np.zeros((NT, 8, 128, CH), np.float64)
    for rmap in res.results:
        acc += rmap["out"].astype(np.float64)
    partialT = acc.transpose(1, 2, 0, 3).reshape(C, T)  # [o, t]
    full = partialT.T + np.asarray(proj_b, np.float64)[None, :]
    if _trace:
        _CACHE["last_result"] = res
    return full.astype(np.float32)


# revision 13
# speedup vs baseline: 1.1364x; 1.1364x over previous
"""Causal self-attention (T=2048, C=1024, H=16) on 8 trn2 NeuronCores.

Tensor-parallel over heads: core i computes heads 2i, 2i+1 (q/k/v rows
128i:128i+128 of each 1024-row block of wqkv_w, proj_w columns
128i:128i+128), producing a partial output projection; partials are summed
on the host (the all-reduce of the sharding hint).

Per-core Bass/Tile kernel, bf16 matmuls with fp32 PSUM accumulation.
Key structure (v2):
  B. qkvT[j, t] = wqkv.T @ xT, contraction-tile outer so matmuls chase the
     x DMAs (x tiles spread over 3 DMA queues); q/k psum held as [128,2,CH]
     2-bank tiles so each evacuation is one [128,1024] DVE op. v's PE
     transposes are full 128x128 blocks (both heads at once) feeding
     v_aug[k, 65] (ones column = softmax denominator via the PV matmul).
  D. per 512-col t-chunk, k-tiles in PAIRS: both scores of a pair land in
     one [128,2,CH] 2-bank PSUM tile, one [128,<=1024] exp ACTIVATE per
     pair per head (halves ScalarE instruction overhead; the two heads'
     score matmuls row-tile concurrently on the PE via base_partition 0/64).
     Causal affine_select on gpsimd (diagonal k-tiles only); PV pipelined
     behind the scores; the previous chunk's normalize/proj matmuls spread
     through the pair-loop as PE filler.
     Normalize: 1/sums on the DVE (vector.reciprocal on the [1,1024] sums
     row - no ScalarE Ln/Exp, no act-table switches), partition-broadcast
     via K=1 float32r matmul with ones, one DVE multiply per head writing
     into the shared attn[128, T] tile (head h at partitions 64h:64h+64).
  E. partialT[o, t] = projT.T @ attn: ONE K=128 matmul per 128-col o-tile
     (both heads contracted together), evacuated bf16 and stored as
     contiguous 128KB DMAs on rotating queues.
"""

import sys

if "/opt/trn_rl_repo" not in sys.path:
    sys.path.insert(0, "/opt/trn_rl_repo")

import ml_dtypes
import numpy as np

T = 2048
C = 1024
CH = 512  # t-chunk width (one PSUM bank of fp32)
NT = T // CH  # 4 t-chunks
NK = T // 128  # 16 k-tiles
NCT = C // 128  # 8 contraction tiles
N_CORES = 8
PIPE = 4  # scores->PV pipeline depth in k-tile steps

_CACHE = {}


def _patch_act_tables(bacc_mod, mybir):
    """Make Exp and Ln resolve to the one table set containing both, so the
    kernel needs a single ACT_TABLE_LOAD instead of thrashing between
    exp_and_others and natural_log_exp_and_others (~1.3us per reload)."""
    if getattr(bacc_mod, "_attn_act_patch", False):
        return
    orig = bacc_mod.get_activation_tables
    both = {mybir.ActivationFunctionType.Exp, mybir.ActivationFunctionType.Ln}

    def patched(arch):
        tabs = dict(orig(arch))
        return {
            name: (funcs if name == "natural_log_exp_and_others" else funcs - both)
            for name, funcs in tabs.items()
        }

    bacc_mod.get_activation_tables = patched
    bacc_mod._attn_act_patch = True


def _build():
    import concourse.tile as tile
    from concourse import bacc, mybir

    _patch_act_tables(bacc, mybir)

    F32 = mybir.dt.float32
    F32R = mybir.dt.float32r
    BF16 = mybir.dt.bfloat16
    EXP = mybir.ActivationFunctionType.Exp
    LN = mybir.ActivationFunctionType.Ln
    IS_GE = mybir.AluOpType.is_ge

    nc = bacc.Bacc(
        "TRN2",
        target_bir_lowering=False,
        debug=False,
        enable_asserts=False,
        num_devices=N_CORES,
        num_swdge_queues=4,
    )
    xT = nc.dram_tensor("xT", [C, T], BF16, kind="ExternalInput").ap()
    wqkv = nc.dram_tensor("wqkv", [C, 384], BF16, kind="ExternalInput").ap()
    projT = nc.dram_tensor("projT", [128, C], BF16, kind="ExternalInput").ap()
    identb = nc.dram_tensor("identb", [128, 128], BF16, kind="ExternalInput").ap()
    ones_f = nc.dram_tensor("ones_f", [128, 128], F32R, kind="ExternalInput").ap()
    bias = nc.dram_tensor("bias", [128, 3], F32, kind="ExternalInput").ap()
    # output as contiguous [chunk, o-tile, 128, 512] bf16 tiles: each store is
    # one fully-contiguous 128KB DMA
    out = nc.dram_tensor("out", [NT, 8, 128, CH], BF16, kind="ExternalOutput").ap()

    with tile.TileContext(nc) as tc:
        with (
            tc.tile_pool(name="big", bufs=1) as big,
            tc.tile_pool(name="expw", bufs=6) as expw_pool,
            tc.tile_pool(name="attn_tmp", bufs=2) as attn_tmp_pool,
            tc.tile_pool(name="outev", bufs=3) as outev_pool,
            tc.tile_pool(name="ps", bufs=1, space="PSUM") as ps,
        ):
            # ---- resident SBUF tensors -------------------------------------
            x_sb = big.tile([128, NCT, T], BF16, name="x_sb")
            w_sb = big.tile([128, NCT, 384], BF16, name="w_sb")
            projT_sb = big.tile([128, C], BF16, name="projT_sb")
            qT_sb = big.tile([128, T], BF16, name="qT_sb")
            kT_sb = big.tile([128, T], BF16, name="kT_sb")
            vT_sb = big.tile([128, T], BF16, name="vT_sb")
            v_aug0 = big.tile([128, NK, 65], BF16, name="v_aug0")
            v_aug1 = big.tile([128, NK, 65], BF16, name="v_aug1")
            attn_sb = big.tile([128, T], BF16, name="attn_sb")
            ident_sb = big.tile([128, 128], BF16, name="ident_sb")
            ones_sb = big.tile([128, 128], F32R, name="ones_sb")
            bias_sb = big.tile([128, 3], F32, name="bias_sb")

            # x and w tiles are fully-contiguous DRAM regions (full rows), so
            # each DMA streams at queue peak; x tiles spread across 3 queues
            # (scalar HWDGE, gpsimd SWDGE, sync HWDGE - all idle this early)
            # so the 4MB load isn't bottlenecked on one queue's rate.
            xengs = [nc.scalar] * 3 + [nc.gpsimd] * 3 + [nc.sync] * 2
            for ct in range(NCT):
                nc.sync.dma_start(
                    out=w_sb[:, ct, :], in_=wqkv[128 * ct : 128 * ct + 128, :]
                )
                xengs[ct].dma_start(
                    out=x_sb[:, ct, :], in_=xT[128 * ct : 128 * ct + 128, :]
                )
            nc.sync.dma_start(out=bias_sb, in_=bias)
            nc.sync.dma_start(out=ident_sb, in_=identb)
            nc.sync.dma_start(out=ones_sb, in_=ones_f)
            nc.sync.dma_start(out=projT_sb, in_=projT)

            nc.vector.memset(v_aug0[:, :, 64:65], 1.0)
            nc.vector.memset(v_aug1[:, :, 64:65], 1.0)

            # ---- stage B: q/k projections, two ct-outer sweeps -------------
            # part 0 = q, part 1 = k; one [128, 2, CH] psum tile per part
            # holds the two 512-col t-chunks of this sweep.
            for sweep in range(2):
                grp = {}
                for part in (0, 1):
                    grp[part] = ps.tile(
                        [128, 2, CH], F32, tag="s2", bufs=2, name=f"qkps_{sweep}_{part}"
                    )
                for ct in range(NCT):
                    for part, g in grp.items():
                        cols = slice(128 * part, 128 * part + 128)
                        for half in (0, 1):
                            c = 2 * sweep + half
                            nc.tensor.matmul(
                                g[:, half, :],
                                w_sb[:, ct, cols],
                                x_sb[:, ct, CH * c : CH * c + CH],
                                start=(ct == 0),
                                stop=(ct == NCT - 1),
                            )
                for part, g in grp.items():
                    dest = qT_sb if part == 0 else kT_sb
                    nc.vector.tensor_scalar_add(
                        dest[:, CH * 2 * sweep : CH * 2 * sweep + 2 * CH],
                        g,
                        bias_sb[:, part : part + 1],
                    )

            # v projection + PE transposes, per chunk; chunks 0,1 up front,
            # the rest emitted as PE filler inside stage D's pair-loops.
            def emit_v_chunk(c):
                v_ps = ps.tile([128, CH], F32, tag="m", bufs=2, name=f"vps_{c}")
                for ct in range(NCT):
                    nc.tensor.matmul(
                        v_ps,
                        w_sb[:, ct, 256:384],
                        x_sb[:, ct, CH * c : CH * c + CH],
                        start=(ct == 0),
                        stop=(ct == NCT - 1),
                    )
                nc.vector.tensor_scalar_add(
                    vT_sb[:, CH * c : CH * c + CH], v_ps, bias_sb[:, 2:3]
                )

            def transposes_for(c):
                # full 128x128 transposes: both heads' v in one shot
                for kt in range(4 * c, 4 * c + 4):
                    tr_ps = ps.tile([128, 128], BF16, tag="m", bufs=2, name=f"tr_{kt}")
                    nc.tensor.transpose(
                        tr_ps, vT_sb[:, 128 * kt : 128 * kt + 128], ident_sb
                    )
                    nc.vector.tensor_copy(v_aug0[:, kt, 0:64], tr_ps[:, 0:64])
                    nc.vector.tensor_copy(v_aug1[:, kt, 0:64], tr_ps[:, 64:128])

            emit_v_chunk(0)
            transposes_for(0)
            emit_v_chunk(1)
            transposes_for(1)

            # ---- stages D+E per t-chunk ------------------------------------
            # Deferred work from chunk c-1, spread through chunk c's pair-loop.
            pending_norm = None  # (at2, chunk)
            pending_proj = None  # chunk index

            def emit_norm(at2, pc):
                tcol = slice(CH * pc, CH * pc + CH)
                for h in (0, 1):
                    rb_ps = ps.tile([64, CH], F32, tag="m", bufs=2, name=f"rb_{h}_{pc}")
                    nc.tensor.matmul(
                        rb_ps,
                        ones_sb[64:65, 0:64],
                        at2[64:65, h, :],
                        start=True,
                        stop=True,
                    )
                    nc.vector.tensor_mul(
                        attn_sb[64 * h : 64 * h + 64, tcol], at2[0:64, h, :], rb_ps
                    )

            store_engs = [nc.sync, nc.sync, nc.sync, nc.gpsimd]

            def emit_proj_tile(pc, m, scalar_evac=False):
                tcol = slice(CH * pc, CH * pc + CH)
                pr_ps = ps.tile([128, CH], F32, tag="m", bufs=2, name=f"pr_{m}_{pc}")
                nc.tensor.matmul(
                    pr_ps,
                    projT_sb[:, 128 * m : 128 * m + 128],
                    attn_sb[:, tcol],
                    start=True,
                    stop=True,
                )
                ob = outev_pool.tile([128, CH], BF16, tag="outev", name=f"ob_{m}_{pc}")
                if scalar_evac:
                    nc.scalar.copy(ob, pr_ps)
                else:
                    nc.vector.tensor_copy(ob, pr_ps)
                store_engs[m % 4].dma_start(out=out[pc, m], in_=ob)

            # Chunk order (1, 2, 3, 0): the ScalarE-heavy late chunks get the
            # previous chunk's projection matmuls as PE filler, and the final
            # chunk processed (0) has the shortest tail.
            for c in (1, 2, 3, 0):
                npair = 2 * c + 2
                pv_ps = {
                    h: ps.tile([65, CH], F32, tag="pv", bufs=2, name=f"pv_{h}_{c}")
                    for h in (0, 1)
                }
                pending = []
                proj_emitted = 0

                def emit_pv(step):
                    for (pj, ph, pw, plane, plo) in step:
                        nc.tensor.matmul(
                            pv_ps[ph][:, plo:CH],
                            (v_aug0 if ph == 0 else v_aug1)[:, pj, :],
                            pw[:, plane, plo:CH],
                            start=(pj == 0),
                            stop=(pj == 4 * c + 3),
                        )

                for P in range(npair):
                    for h in (0, 1):
                        hrow = slice(64 * h, 64 * h + 64)
                        s2 = ps.tile(
                            [128, 2, CH], F32, tag="s2", bufs=2, name=f"s_{h}_{c}_{P}"
                        )
                        # pair-level column offset: both planes computed from
                        # slo_pair so the pair exp reads only written PSUM
                        # (the second diagonal plane recomputes 128 masked
                        # cols - 128 cycles, cheaper than a separate ACT)
                        slo_pair = max(0, 128 * (2 * P - 4 * c))
                        for i in (0, 1):
                            j = 2 * P + i
                            nc.tensor.matmul(
                                s2[:, i, slo_pair:CH],
                                kT_sb[hrow, 128 * j : 128 * j + 128],
                                qT_sb[hrow, CH * c + slo_pair : CH * c + CH],
                                start=True,
                                stop=True,
                            )
                        w2 = expw_pool.tile(
                            [128, 2, CH], BF16, tag="expw", name=f"w_{h}_{c}_{P}"
                        )
                        nc.scalar.activation(
                            out=w2[:, :, slo_pair:CH],
                            in_=s2[:, :, slo_pair:CH],
                            func=EXP,
                        )
                        for i in (0, 1):
                            j = 2 * P + i
                            diag = j - 4 * c
                            if diag >= 0:
                                # keep exp(score) where t >= k: within the
                                # kept column range f' = f - 128*diag, so
                                # f' - p >= 0
                                lo = 128 * diag if diag > 0 else 0
                                nc.gpsimd.affine_select(
                                    out=w2[:, i, lo:CH],
                                    in_=w2[:, i, lo:CH],
                                    pattern=[[1, CH - lo]],
                                    compare_op=IS_GE,
                                    fill=0.0,
                                    base=0,
                                    channel_multiplier=-1,
                                )
                            pending.append((j, h, w2, i, max(0, 128 * diag)))
                    if P == 0 and pending_norm is not None:
                        emit_norm(*pending_norm)
                        pending_norm = None
                    if P == 1 and c == 1:
                        emit_v_chunk(2)
                    if P == 2 and c == 1:
                        transposes_for(2)
                    if P == 1 and c == 3:
                        emit_v_chunk(3)
                    if P == 3 and c == 3:
                        transposes_for(3)
                    while len(pending) > 2 * PIPE:
                        step, pending = pending[:2], pending[2:]
                        emit_pv(step)
                    if pending_proj is not None and P >= 1:
                        target = (P * 8) // max(npair - 1, 1)
                        while proj_emitted < min(target, 8):
                            emit_proj_tile(pending_proj, proj_emitted)
                            proj_emitted += 1
                while pending:
                    step, pending = pending[:2], pending[2:]
                    emit_pv(step)
                if pending_proj is not None:
                    while proj_emitted < 8:
                        emit_proj_tile(pending_proj, proj_emitted)
                        proj_emitted += 1

                at2 = attn_tmp_pool.tile(
                    [65, 2, CH], F32R, tag="attn_tmp", name=f"at2_{c}"
                )
                for h in (0, 1):
                    nc.vector.tensor_copy(at2[:, h, :], pv_ps[h])
                # 1/sums = exp(-ln(sums)) on ScalarE; Ln+Exp share one act
                # table set (DVE reciprocal measures ~6.4ns/elem on a [1,1024]
                # row - far slower than two ScalarE activations)
                rrow = at2[64:65, :, :]
                nc.scalar.activation(out=rrow, in_=rrow, func=LN)
                nc.scalar.activation(out=rrow, in_=rrow, func=EXP, scale=-1.0)
                pending_norm = (at2, c)
                pending_proj = c

            # tail: ScalarE is done with exps - let it evacuate half the
            # final chunk's proj tiles in parallel with the DVE
            emit_norm(*pending_norm)
            for m in range(8):
                emit_proj_tile(pending_proj, m, scalar_evac=(m % 2 == 1))

    nc.compile()
    return nc


def _get_nc():
    if "nc" not in _CACHE:
        _CACHE["nc"] = _build()
    return _CACHE["nc"]


def _make_in_maps(x, wqkv_w, wqkv_b, proj_w):
    bf = ml_dtypes.bfloat16
    xT = np.ascontiguousarray(np.asarray(x, np.float32).T.astype(bf))
    identb = np.eye(128, dtype=bf)
    ones_f = np.ones((128, 128), np.float32)
    scale = np.float32(1.0 / np.sqrt(C))
    in_maps = []
    for i in range(N_CORES):
        rows = []
        biases = []
        for blk, s in ((0, scale), (1, None), (2, None)):
            sl = slice(blk * C + 128 * i, blk * C + 128 * i + 128)
            w = np.asarray(wqkv_w[sl], np.float32)
            b = np.asarray(wqkv_b[sl], np.float32)
            if s is not None:
                w = w * s
                b = b * s
            rows.append(w)
            biases.append(b)
        W = np.concatenate(rows, axis=0)  # [384, 1024]
        B = np.stack(biases, axis=1)  # [128, 3]
        pT = np.asarray(proj_w[:, 128 * i : 128 * i + 128], np.float32).T  # [128, 1024]
        in_maps.append(
            {
                "xT": xT,
                "wqkv": np.ascontiguousarray(W.T.astype(bf)),
                "projT": np.ascontiguousarray(pT.astype(bf)),
                "identb": identb,
                "ones_f": ones_f,
                "bias": np.ascontiguousarray(B),
            }
        )
    return in_maps


def kernel(x, wqkv_w, wqkv_b, proj_w, proj_b, _trace=False, _tmpdir=None):
    from concourse.bass_utils import run_bass_kernel_spmd

    nc = _get_nc()
    in_maps = _make_in_maps(x, wqkv_w, wqkv_b, proj_w)
    res = run_bass_kernel_spmd(
        nc,
        in_maps,
        core_ids=list(range(N_CORES)),
        trace=_trace,
        tmpdir=_tmpdir,
    )
    acc = np.zeros((NT, 8, 128, CH), np.float64)
    for rmap in res.results:
        acc += rmap["out"].astype(np.float64)
    partialT = acc.transpose(1, 2, 0, 3).reshape(C, T)  # [o, t]
    full = partialT.T + np.asarray(proj_b, np.float64)[None, :]
    if _trace:
        _CACHE["last_result"] = res
    return full.astype(np.float32)


# revision 20
# speedup vs baseline: 1.2034x; 1.0590x over previous
"""Causal self-attention (T=2048, C=1024, H=16) on 8 trn2 NeuronCores.

Tensor-parallel over heads: core i computes heads 2i, 2i+1 (q/k/v rows
128i:128i+128 of each 1024-row block of wqkv_w, proj_w columns
128i:128i+128), producing a partial output projection; partials are summed
on the host (the all-reduce of the sharding hint).

Per-core Bass/Tile kernel, bf16 matmuls with fp32 PSUM accumulation.
Key structure (v2):
  B. qkvT[j, t] = wqkv.T @ xT, contraction-tile outer so matmuls chase the
     x DMAs (x tiles spread over 3 DMA queues); q/k psum held as [128,2,CH]
     2-bank tiles so each evacuation is one [128,1024] DVE op. v's PE
     transposes are full 128x128 blocks (both heads at once) feeding
     v_aug[k, 65] (ones column = softmax denominator via the PV matmul).
  D. per 512-col t-chunk, k-tiles in PAIRS: both scores of a pair land in
     one [128,2,CH] 2-bank PSUM tile, one [128,<=1024] exp ACTIVATE per
     pair per head (halves ScalarE instruction overhead; the two heads'
     score matmuls row-tile concurrently on the PE via base_partition 0/64).
     Causal affine_select on gpsimd (diagonal k-tiles only); PV pipelined
     behind the scores; the previous chunk's normalize/proj matmuls spread
     through the pair-loop as PE filler.
     Normalize: 1/sums on the DVE (vector.reciprocal on the [1,1024] sums
     row - no ScalarE Ln/Exp, no act-table switches), partition-broadcast
     via K=1 float32r matmul with ones, one DVE multiply per head writing
     into the shared attn[128, T] tile (head h at partitions 64h:64h+64).
  E. partialT[o, t] = projT.T @ attn: ONE K=128 matmul per 128-col o-tile
     (both heads contracted together), evacuated bf16 and stored as
     contiguous 128KB DMAs on rotating queues.
"""

import sys

if "/opt/trn_rl_repo" not in sys.path:
    sys.path.insert(0, "/opt/trn_rl_repo")

import ml_dtypes
import numpy as np

T = 2048
C = 1024
CH = 512  # t-chunk width (one PSUM bank of fp32)
NT = T // CH  # 4 t-chunks
NK = T // 128  # 16 k-tiles
NCT = C // 128  # 8 contraction tiles
N_CORES = 8
PIPE = 4  # scores->PV pipeline depth in k-tile steps

_CACHE = {}


def _patch_act_tables(bacc_mod, mybir):
    """Make Exp and Ln resolve to the one table set containing both, so the
    kernel needs a single ACT_TABLE_LOAD instead of thrashing between
    exp_and_others and natural_log_exp_and_others (~1.3us per reload)."""
    if getattr(bacc_mod, "_attn_act_patch", False):
        return
    orig = bacc_mod.get_activation_tables
    both = {mybir.ActivationFunctionType.Exp, mybir.ActivationFunctionType.Ln}

    def patched(arch):
        tabs = dict(orig(arch))
        return {
            name: (funcs if name == "natural_log_exp_and_others" else funcs - both)
            for name, funcs in tabs.items()
        }

    bacc_mod.get_activation_tables = patched
    bacc_mod._attn_act_patch = True


def _build():
    import concourse.tile as tile
    from concourse import bacc, mybir

    _patch_act_tables(bacc, mybir)

    F32 = mybir.dt.float32
    F32R = mybir.dt.float32r
    BF16 = mybir.dt.bfloat16
    EXP = mybir.ActivationFunctionType.Exp
    LN = mybir.ActivationFunctionType.Ln
    IS_GE = mybir.AluOpType.is_ge

    nc = bacc.Bacc(
        "TRN2",
        target_bir_lowering=False,
        debug=False,
        enable_asserts=False,
        num_devices=N_CORES,
        num_swdge_queues=4,
    )
    xT = nc.dram_tensor("xT", [C, T], BF16, kind="ExternalInput").ap()
    wqkv = nc.dram_tensor("wqkv", [C, 384], BF16, kind="ExternalInput").ap()
    projT = nc.dram_tensor("projT", [128, C], BF16, kind="ExternalInput").ap()
    identb = nc.dram_tensor("identb", [128, 128], BF16, kind="ExternalInput").ap()
    ones_f = nc.dram_tensor("ones_f", [128, 128], F32R, kind="ExternalInput").ap()
    bias = nc.dram_tensor("bias", [128, 3], F32, kind="ExternalInput").ap()
    # output as contiguous [chunk, o-tile, 128, 512] bf16 tiles: each store is
    # one fully-contiguous 128KB DMA
    out = nc.dram_tensor("out", [NT, 8, 128, CH], BF16, kind="ExternalOutput").ap()

    with tile.TileContext(nc) as tc:
        with (
            tc.tile_pool(name="big", bufs=1) as big,
            tc.tile_pool(name="expw", bufs=6) as expw_pool,
            tc.tile_pool(name="attn_tmp", bufs=2) as attn_tmp_pool,
            tc.tile_pool(name="outev", bufs=3) as outev_pool,
            tc.tile_pool(name="ps", bufs=1, space="PSUM") as ps,
        ):
            # ---- resident SBUF tensors -------------------------------------
            x_sb = big.tile([128, NCT, T], BF16, name="x_sb")
            w_sb = big.tile([128, NCT, 384], BF16, name="w_sb")
            projT_sb = big.tile([128, C], BF16, name="projT_sb")
            qT_sb = big.tile([128, T], BF16, name="qT_sb")
            kT_sb = big.tile([128, T], BF16, name="kT_sb")
            vT_sb = big.tile([128, T], BF16, name="vT_sb")
            v_aug0 = big.tile([128, NK, 65], BF16, name="v_aug0")
            v_aug1 = big.tile([128, NK, 65], BF16, name="v_aug1")
            attn_sb = big.tile([128, T], BF16, name="attn_sb")
            ident_sb = big.tile([128, 128], BF16, name="ident_sb")
            ones_sb = big.tile([128, 128], F32R, name="ones_sb")
            bias_sb = big.tile([128, 3], F32, name="bias_sb")

            # x and w tiles are fully-contiguous DRAM regions (full rows), so
            # each DMA streams at queue peak; x tiles split between the scalar
            # HWDGE and gpsimd SWDGE queues (both idle this early) while sync
            # carries only the small weight/misc tensors.
            xengs = [nc.scalar, nc.gpsimd] * 4
            for ct in range(NCT):
                nc.sync.dma_start(
                    out=w_sb[:, ct, :], in_=wqkv[128 * ct : 128 * ct + 128, :]
                )
                xengs[ct].dma_start(
                    out=x_sb[:, ct, :], in_=xT[128 * ct : 128 * ct + 128, :]
                )
            nc.sync.dma_start(out=bias_sb, in_=bias)
            nc.sync.dma_start(out=ident_sb, in_=identb)
            nc.sync.dma_start(out=ones_sb, in_=ones_f)
            nc.sync.dma_start(out=projT_sb, in_=projT)

            nc.vector.memset(v_aug0[:, :, 64:65], 1.0)
            nc.vector.memset(v_aug1[:, :, 64:65], 1.0)

            # ---- stage B: q/k projections, ct-outer sweeps -----------------
            # part 0 = q, part 1 = k; one [128, 2, CH] psum tile per part
            # holds the two 512-col t-chunks of a sweep. Sweep 0 runs in the
            # prelude (chasing the x DMAs); sweep 1's matmuls are emitted in
            # ct-sized slices as PE filler inside the first D chunks.
            def qk_sweep_open(sweep):
                grp = {}
                for part in (0, 1):
                    grp[part] = ps.tile(
                        [128, 2, CH], F32, tag="s2", bufs=2, name=f"qkps_{sweep}_{part}"
                    )
                return grp

            def qk_sweep_cts(sweep, grp, cts):
                for ct in cts:
                    for part, g in grp.items():
                        cols = slice(128 * part, 128 * part + 128)
                        for half in (0, 1):
                            c = 2 * sweep + half
                            nc.tensor.matmul(
                                g[:, half, :],
                                w_sb[:, ct, cols],
                                x_sb[:, ct, CH * c : CH * c + CH],
                                start=(ct == 0),
                                stop=(ct == NCT - 1),
                            )

            def qk_sweep_close(sweep, grp):
                for part, g in grp.items():
                    dest = qT_sb if part == 0 else kT_sb
                    nc.vector.tensor_scalar_add(
                        dest[:, CH * 2 * sweep : CH * 2 * sweep + 2 * CH],
                        g,
                        bias_sb[:, part : part + 1],
                    )

            grp0 = qk_sweep_open(0)
            qk_sweep_cts(0, grp0, range(NCT))
            qk_sweep_close(0, grp0)

            # v projection + PE transposes, per chunk; chunks 0,1 up front,
            # the rest emitted as PE filler inside stage D's pair-loops.
            def emit_v_chunk(c):
                v_ps = ps.tile([128, CH], F32, tag="m", bufs=2, name=f"vps_{c}")
                for ct in range(NCT):
                    nc.tensor.matmul(
                        v_ps,
                        w_sb[:, ct, 256:384],
                        x_sb[:, ct, CH * c : CH * c + CH],
                        start=(ct == 0),
                        stop=(ct == NCT - 1),
                    )
                nc.vector.tensor_scalar_add(
                    vT_sb[:, CH * c : CH * c + CH], v_ps, bias_sb[:, 2:3]
                )

            def transposes_for(c):
                # full 128x128 transposes: both heads' v in one shot
                for kt in range(4 * c, 4 * c + 4):
                    tr_ps = ps.tile([128, 128], BF16, tag="m", bufs=2, name=f"tr_{kt}")
                    nc.tensor.transpose(
                        tr_ps, vT_sb[:, 128 * kt : 128 * kt + 128], ident_sb
                    )
                    nc.vector.tensor_copy(v_aug0[:, kt, 0:64], tr_ps[:, 0:64])
                    nc.vector.tensor_copy(v_aug1[:, kt, 0:64], tr_ps[:, 64:128])

            emit_v_chunk(0)
            transposes_for(0)

            # ---- stages D+E per t-chunk ------------------------------------
            # Deferred work from chunk c-1, spread through chunk c's pair-loop.
            pending_norm = None  # (at2, chunk)
            pending_proj = None  # chunk index

            def emit_norm(at2, pc):
                tcol = slice(CH * pc, CH * pc + CH)
                for h in (0, 1):
                    rb_ps = ps.tile([64, CH], F32, tag="m", bufs=2, name=f"rb_{h}_{pc}")
                    nc.tensor.matmul(
                        rb_ps,
                        ones_sb[64:65, 0:64],
                        at2[64:65, h, :],
                        start=True,
                        stop=True,
                    )
                    nc.vector.tensor_mul(
                        attn_sb[64 * h : 64 * h + 64, tcol], at2[0:64, h, :], rb_ps
                    )

            store_engs = [nc.sync, nc.gpsimd]

            def emit_proj_tile(pc, m, scalar_evac=False):
                tcol = slice(CH * pc, CH * pc + CH)
                pr_ps = ps.tile([128, CH], F32, tag="m", bufs=2, name=f"pr_{m}_{pc}")
                nc.tensor.matmul(
                    pr_ps,
                    projT_sb[:, 128 * m : 128 * m + 128],
                    attn_sb[:, tcol],
                    start=True,
                    stop=True,
                )
                ob = outev_pool.tile([128, CH], BF16, tag="outev", name=f"ob_{m}_{pc}")
                if scalar_evac:
                    nc.scalar.copy(ob, pr_ps)
                else:
                    nc.vector.tensor_copy(ob, pr_ps)
                store_engs[m % 2].dma_start(out=out[pc, m], in_=ob)

            # q/k sweep-1 sliced into 1-bank [128, CH] sub-sweeps (tag "m")
            # so they interleave with D's scores without touching the s2
            # score-pair buffers.
            def qk_sub(part, c):
                g = ps.tile([128, CH], F32, tag="m", bufs=2, name=f"qks_{part}_{c}")
                cols = slice(128 * part, 128 * part + 128)
                for ct in range(NCT):
                    nc.tensor.matmul(
                        g,
                        w_sb[:, ct, cols],
                        x_sb[:, ct, CH * c : CH * c + CH],
                        start=(ct == 0),
                        stop=(ct == NCT - 1),
                    )
                dest = qT_sb if part == 0 else kT_sb
                nc.vector.tensor_scalar_add(
                    dest[:, CH * c : CH * c + CH], g, bias_sb[:, part : part + 1]
                )

            # fillers[(c, P)] -> list of emit thunks: stage-B leftovers spread
            # through the early chunks' pair loops as PE filler, each slice
            # timed to land before its first consumer.
            fillers = {
                (0, 0): [lambda: emit_v_chunk(1)],
                (0, 1): [lambda: transposes_for(1), lambda: qk_sub(0, 2)],
                (1, 0): [lambda: qk_sub(1, 2)],
                (1, 1): [lambda: emit_v_chunk(2)],
                (1, 2): [lambda: transposes_for(2)],
                (1, 3): [lambda: qk_sub(0, 3)],
                (2, 0): [lambda: qk_sub(1, 3)],
                (2, 1): [lambda: emit_v_chunk(3)],
                (2, 2): [lambda: transposes_for(3)],
            }

            # Chunk order (0, 1, 2, 3): the first chunks' slack absorbs the
            # stage-B leftovers, and the final chunk is the ScalarE-heaviest
            # (8 exp pairs), giving the PE the deepest well of parallel work
            # while only ONE chunk's norm+proj+store tail remains at the end.
            for c in (0, 1, 2, 3):
                npair = 2 * c + 2
                pv_ps = {
                    h: ps.tile([65, CH], F32, tag="pv", bufs=2, name=f"pv_{h}_{c}")
                    for h in (0, 1)
                }
                pending = []
                proj_emitted = 0

                def emit_pv(step):
                    for (pj, ph, pw, plane, plo) in step:
                        nc.tensor.matmul(
                            pv_ps[ph][:, plo:CH],
                            (v_aug0 if ph == 0 else v_aug1)[:, pj, :],
                            pw[:, plane, plo:CH],
                            start=(pj == 0),
                            stop=(pj == 4 * c + 3),
                        )

                for P in range(npair):
                    for h in (0, 1):
                        hrow = slice(64 * h, 64 * h + 64)
                        s2 = ps.tile(
                            [128, 2, CH], F32, tag="s2", bufs=2, name=f"s_{h}_{c}_{P}"
                        )
                        # pair-level column offset: both planes computed from
                        # slo_pair so the pair exp reads only written PSUM
                        # (the second diagonal plane recomputes 128 masked
                        # cols - 128 cycles, cheaper than a separate ACT)
                        slo_pair = max(0, 128 * (2 * P - 4 * c))
                        for i in (0, 1):
                            j = 2 * P + i
                            nc.tensor.matmul(
                                s2[:, i, slo_pair:CH],
                                kT_sb[hrow, 128 * j : 128 * j + 128],
                                qT_sb[hrow, CH * c + slo_pair : CH * c + CH],
                                start=True,
                                stop=True,
                            )
                        w2 = expw_pool.tile(
                            [128, 2, CH], BF16, tag="expw", name=f"w_{h}_{c}_{P}"
                        )
                        nc.scalar.activation(
                            out=w2[:, :, slo_pair:CH],
                            in_=s2[:, :, slo_pair:CH],
                            func=EXP,
                        )
                        for i in (0, 1):
                            j = 2 * P + i
                            diag = j - 4 * c
                            if diag >= 0:
                                # keep exp(score) where t >= k: within the
                                # kept column range f' = f - 128*diag, so
                                # f' - p >= 0
                                lo = 128 * diag if diag > 0 else 0
                                nc.gpsimd.affine_select(
                                    out=w2[:, i, lo:CH],
                                    in_=w2[:, i, lo:CH],
                                    pattern=[[1, CH - lo]],
                                    compare_op=IS_GE,
                                    fill=0.0,
                                    base=0,
                                    channel_multiplier=-1,
                                )
                            pending.append((j, h, w2, i, max(0, 128 * diag)))
                    if P == 0 and pending_norm is not None:
                        emit_norm(*pending_norm)
                        pending_norm = None
                    for thunk in fillers.get((c, P), ()):
                        thunk()
                    while len(pending) > 2 * PIPE:
                        step, pending = pending[:2], pending[2:]
                        emit_pv(step)
                    if pending_proj is not None and P >= 1:
                        target = (P * 8) // max(npair - 1, 1)
                        while proj_emitted < min(target, 8):
                            emit_proj_tile(pending_proj, proj_emitted)
                            proj_emitted += 1
                while pending:
                    step, pending = pending[:2], pending[2:]
                    emit_pv(step)
                if pending_proj is not None:
                    while proj_emitted < 8:
                        emit_proj_tile(pending_proj, proj_emitted)
                        proj_emitted += 1

                at2 = attn_tmp_pool.tile(
                    [65, 2, CH], F32R, tag="attn_tmp", name=f"at2_{c}"
                )
                for h in (0, 1):
                    nc.vector.tensor_copy(at2[:, h, :], pv_ps[h])
                # 1/sums = exp(-ln(sums)) on ScalarE; Ln+Exp share one act
                # table set (DVE reciprocal measures ~6.4ns/elem on a [1,1024]
                # row - far slower than two ScalarE activations)
                rrow = at2[64:65, :, :]
                nc.scalar.activation(out=rrow, in_=rrow, func=LN)
                nc.scalar.activation(out=rrow, in_=rrow, func=EXP, scale=-1.0)
                pending_norm = (at2, c)
                pending_proj = c

            # tail: ScalarE is done with exps - let it evacuate half the
            # final chunk's proj tiles in parallel with the DVE
            emit_norm(*pending_norm)
            for m in range(8):
                emit_proj_tile(pending_proj, m, scalar_evac=(m % 2 == 1))

    nc.compile()
    return nc


def _get_nc():
    if "nc" not in _CACHE:
        _CACHE["nc"] = _build()
    return _CACHE["nc"]


def _make_in_maps(x, wqkv_w, wqkv_b, proj_w):
    bf = ml_dtypes.bfloat16
    xT = np.ascontiguousarray(np.asarray(x, np.float32).T.astype(bf))
    identb = np.eye(128, dtype=bf)
    ones_f = np.ones((128, 128), np.float32)
    scale = np.float32(1.0 / np.sqrt(C))
    in_maps = []
    for i in range(N_CORES):
        rows = []
        biases = []
        for blk, s in ((0, scale), (1, None), (2, None)):
            sl = slice(blk * C + 128 * i, blk * C + 128 * i + 128)
            w = np.asarray(wqkv_w[sl], np.float32)
            b = np.asarray(wqkv_b[sl], np.float32)
            if s is not None:
                w = w * s
                b = b * s
            rows.append(w)
            biases.append(b)
        W = np.concatenate(rows, axis=0)  # [384, 1024]
        B = np.stack(biases, axis=1)  # [128, 3]
        pT = np.asarray(proj_w[:, 128 * i : 128 * i + 128], np.float32).T  # [128, 1024]
        in_maps.append(
            {
                "xT": xT,
                "wqkv": np.ascontiguousarray(W.T.astype(bf)),
                "projT": np.ascontiguousarray(pT.astype(bf)),
                "identb": identb,
                "ones_f": ones_f,
                "bias": np.ascontiguousarray(B),
            }
        )
    return in_maps


def kernel(x, wqkv_w, wqkv_b, proj_w, proj_b, _trace=False, _tmpdir=None):
    from concourse.bass_utils import run_bass_kernel_spmd

    nc = _get_nc()
    in_maps = _make_in_maps(x, wqkv_w, wqkv_b, proj_w)
    res = run_bass_kernel_spmd(
        nc,
        in_maps,
        core_ids=list(range(N_CORES)),
        trace=_trace,
        tmpdir=_tmpdir,
    )
    acc = np.zeros((NT, 8, 128, CH), np.float64)
    for rmap in res.results:
        acc += rmap["out"].astype(np.float64)
    partialT = acc.transpose(1, 2, 0, 3).reshape(C, T)  # [o, t]
    full = partialT.T + np.asarray(proj_b, np.float64)[None, :]
    if _trace:
        _CACHE["last_result"] = res
    return full.astype(np.float32)


# revision 23
# speedup vs baseline: 1.2663x; 1.0523x over previous
"""Causal self-attention (T=2048, C=1024, H=16) on 8 trn2 NeuronCores.

Tensor-parallel over heads: core i computes heads 2i, 2i+1 (q/k/v rows
128i:128i+128 of each 1024-row block of wqkv_w, proj_w columns
128i:128i+128), producing a partial output projection; partials are summed
on the host (the all-reduce of the sharding hint).

Per-core Bass/Tile kernel, bf16 matmuls with fp32 PSUM accumulation.
Key structure (v2):
  B. qkvT[j, t] = wqkv.T @ xT, contraction-tile outer so matmuls chase the
     x DMAs (x tiles spread over 3 DMA queues); q/k psum held as [128,2,CH]
     2-bank tiles so each evacuation is one [128,1024] DVE op. v's PE
     transposes are full 128x128 blocks (both heads at once) feeding
     v_aug[k, 65] (ones column = softmax denominator via the PV matmul).
  D. per 512-col t-chunk, k-tiles in PAIRS: both scores of a pair land in
     one [128,2,CH] 2-bank PSUM tile, one [128,<=1024] exp ACTIVATE per
     pair per head (halves ScalarE instruction overhead; the two heads'
     score matmuls row-tile concurrently on the PE via base_partition 0/64).
     Causal affine_select on gpsimd (diagonal k-tiles only); PV pipelined
     behind the scores; the previous chunk's normalize/proj matmuls spread
     through the pair-loop as PE filler.
     Normalize: 1/sums on the DVE (vector.reciprocal on the [1,1024] sums
     row - no ScalarE Ln/Exp, no act-table switches), partition-broadcast
     via K=1 float32r matmul with ones, one DVE multiply per head writing
     into the shared attn[128, T] tile (head h at partitions 64h:64h+64).
  E. partialT[o, t] = projT.T @ attn: ONE K=128 matmul per 128-col o-tile
     (both heads contracted together), evacuated bf16 and stored as
     contiguous 128KB DMAs on rotating queues.
"""

import sys

if "/opt/trn_rl_repo" not in sys.path:
    sys.path.insert(0, "/opt/trn_rl_repo")

import ml_dtypes
import numpy as np

T = 2048
C = 1024
CH = 512  # t-chunk width (one PSUM bank of fp32)
NT = T // CH  # 4 t-chunks
NK = T // 128  # 16 k-tiles
NCT = C // 128  # 8 contraction tiles
N_CORES = 8
PIPE = 4  # scores->PV pipeline depth in k-tile steps

_CACHE = {}


def _patch_act_tables(bacc_mod, mybir):
    """Make Exp and Ln resolve to the one table set containing both, so the
    kernel needs a single ACT_TABLE_LOAD instead of thrashing between
    exp_and_others and natural_log_exp_and_others (~1.3us per reload)."""
    if getattr(bacc_mod, "_attn_act_patch", False):
        return
    orig = bacc_mod.get_activation_tables
    both = {mybir.ActivationFunctionType.Exp, mybir.ActivationFunctionType.Ln}

    def patched(arch):
        tabs = dict(orig(arch))
        return {
            name: (funcs if name == "natural_log_exp_and_others" else funcs - both)
            for name, funcs in tabs.items()
        }

    bacc_mod.get_activation_tables = patched
    bacc_mod._attn_act_patch = True


def _build():
    import concourse.tile as tile
    from concourse import bacc, mybir

    _patch_act_tables(bacc, mybir)

    F32 = mybir.dt.float32
    F32R = mybir.dt.float32r
    BF16 = mybir.dt.bfloat16
    EXP = mybir.ActivationFunctionType.Exp
    LN = mybir.ActivationFunctionType.Ln
    IS_GE = mybir.AluOpType.is_ge

    nc = bacc.Bacc(
        "TRN2",
        target_bir_lowering=False,
        debug=False,
        enable_asserts=False,
        num_devices=N_CORES,
        num_swdge_queues=4,
    )
    xT = nc.dram_tensor("xT", [C, T], BF16, kind="ExternalInput").ap()
    wqkv = nc.dram_tensor("wqkv", [C, 384], BF16, kind="ExternalInput").ap()
    projT = nc.dram_tensor("projT", [128, C], BF16, kind="ExternalInput").ap()
    identb = nc.dram_tensor("identb", [128, 128], BF16, kind="ExternalInput").ap()
    ones_f = nc.dram_tensor("ones_f", [128, 128], F32R, kind="ExternalInput").ap()
    bias = nc.dram_tensor("bias", [128, 3], F32, kind="ExternalInput").ap()
    # output as contiguous [chunk, o-tile, 128, 512] bf16 tiles: each store is
    # one fully-contiguous 128KB DMA
    out = nc.dram_tensor("out", [NT, 8, 128, CH], BF16, kind="ExternalOutput").ap()

    with tile.TileContext(nc) as tc:
        with (
            tc.tile_pool(name="big", bufs=1) as big,
            tc.tile_pool(name="expw", bufs=6) as expw_pool,
            tc.tile_pool(name="attn_tmp", bufs=2) as attn_tmp_pool,
            tc.tile_pool(name="outev", bufs=3) as outev_pool,
            tc.tile_pool(name="ps", bufs=1, space="PSUM") as ps,
        ):
            # ---- resident SBUF tensors -------------------------------------
            x_sb = big.tile([128, NCT, T], BF16, name="x_sb")
            w_sb = big.tile([128, NCT, 384], BF16, name="w_sb")
            projT_sb = big.tile([128, C], BF16, name="projT_sb")
            qT_sb = big.tile([128, T], BF16, name="qT_sb")
            kT_sb = big.tile([128, T], BF16, name="kT_sb")
            vT_sb = big.tile([128, T], BF16, name="vT_sb")
            v_aug0 = big.tile([128, NK, 65], BF16, name="v_aug0")
            v_aug1 = big.tile([128, NK, 65], BF16, name="v_aug1")
            attn_sb = big.tile([128, T], BF16, name="attn_sb")
            ident_sb = big.tile([128, 128], BF16, name="ident_sb")
            ones_sb = big.tile([128, 128], F32R, name="ones_sb")
            bias_sb = big.tile([128, 3], F32, name="bias_sb")

            # x and w tiles are fully-contiguous DRAM regions (full rows), so
            # each DMA streams at queue peak; x tiles split between the scalar
            # HWDGE and gpsimd SWDGE queues (both idle this early) while sync
            # carries only the small weight/misc tensors.
            xengs = [nc.scalar, nc.gpsimd, nc.sync]
            for ct in range(NCT):
                nc.sync.dma_start(
                    out=w_sb[:, ct, :], in_=wqkv[128 * ct : 128 * ct + 128, :]
                )
            for ct in range(NCT):
                xengs[ct % 3].dma_start(
                    out=x_sb[:, ct, :], in_=xT[128 * ct : 128 * ct + 128, :]
                )
            nc.scalar.dma_start(out=projT_sb, in_=projT)
            nc.sync.dma_start(out=bias_sb, in_=bias)
            nc.sync.dma_start(out=ident_sb, in_=identb)
            nc.sync.dma_start(out=ones_sb, in_=ones_f)

            nc.vector.memset(v_aug0[:, :, 64:65], 1.0)
            nc.vector.memset(v_aug1[:, :, 64:65], 1.0)

            # ---- stage B: q/k projections, ct-outer sweeps -----------------
            # part 0 = q, part 1 = k; one [128, 2, CH] psum tile per part
            # holds the two 512-col t-chunks of a sweep. Sweep 0 runs in the
            # prelude (chasing the x DMAs); sweep 1's matmuls are emitted in
            # ct-sized slices as PE filler inside the first D chunks.
            def qk_sweep_open(sweep):
                grp = {}
                for part in (0, 1):
                    grp[part] = ps.tile(
                        [128, 2, CH], F32, tag="s2", bufs=2, name=f"qkps_{sweep}_{part}"
                    )
                return grp

            def qk_sweep_cts(sweep, grp, cts):
                for ct in cts:
                    for part, g in grp.items():
                        cols = slice(128 * part, 128 * part + 128)
                        for half in (0, 1):
                            c = 2 * sweep + half
                            nc.tensor.matmul(
                                g[:, half, :],
                                w_sb[:, ct, cols],
                                x_sb[:, ct, CH * c : CH * c + CH],
                                start=(ct == 0),
                                stop=(ct == NCT - 1),
                            )

            def qk_sweep_close(sweep, grp):
                for part, g in grp.items():
                    dest = qT_sb if part == 0 else kT_sb
                    nc.vector.tensor_scalar_add(
                        dest[:, CH * 2 * sweep : CH * 2 * sweep + 2 * CH],
                        g,
                        bias_sb[:, part : part + 1],
                    )

            grp0 = qk_sweep_open(0)
            qk_sweep_cts(0, grp0, range(NCT))
            qk_sweep_close(0, grp0)

            # v projection + PE transposes, per chunk; chunks 0,1 up front,
            # the rest emitted as PE filler inside stage D's pair-loops.
            def emit_v_chunk(c):
                v_ps = ps.tile([128, CH], F32, tag="m", bufs=2, name=f"vps_{c}")
                for ct in range(NCT):
                    nc.tensor.matmul(
                        v_ps,
                        w_sb[:, ct, 256:384],
                        x_sb[:, ct, CH * c : CH * c + CH],
                        start=(ct == 0),
                        stop=(ct == NCT - 1),
                    )
                nc.vector.tensor_scalar_add(
                    vT_sb[:, CH * c : CH * c + CH], v_ps, bias_sb[:, 2:3]
                )

            def transposes_for(c):
                # full 128x128 transposes: both heads' v in one shot
                for kt in range(4 * c, 4 * c + 4):
                    tr_ps = ps.tile([128, 128], BF16, tag="m", bufs=2, name=f"tr_{kt}")
                    nc.tensor.transpose(
                        tr_ps, vT_sb[:, 128 * kt : 128 * kt + 128], ident_sb
                    )
                    nc.vector.tensor_copy(v_aug0[:, kt, 0:64], tr_ps[:, 0:64])
                    nc.vector.tensor_copy(v_aug1[:, kt, 0:64], tr_ps[:, 64:128])

            emit_v_chunk(0)
            transposes_for(0)

            # ---- stages D+E per t-chunk ------------------------------------
            # Deferred work from chunk c-1, spread through chunk c's pair-loop:
            # P0: at2 copies + Ln/Exp reciprocal (fills the ScalarE slot that
            # used to be an idle chunk-boundary gap), P2: normalize muls,
            # P>=3: projection tiles.
            pending_fin = None  # (pv_ps dict, chunk)
            pending_norm = None  # (at2, chunk)
            pending_proj = None  # chunk index

            def emit_fin(pv_prev, pc):
                at2 = attn_tmp_pool.tile(
                    [65, 2, CH], F32R, tag="attn_tmp", name=f"at2_{pc}"
                )
                for h in (0, 1):
                    nc.vector.tensor_copy(at2[:, h, :], pv_prev[h])
                # 1/sums = exp(-ln(sums)); Ln+Exp share one act table set
                rrow = at2[64:65, :, :]
                nc.scalar.activation(out=rrow, in_=rrow, func=LN)
                nc.scalar.activation(out=rrow, in_=rrow, func=EXP, scale=-1.0)
                return at2

            def emit_norm(at2, pc):
                tcol = slice(CH * pc, CH * pc + CH)
                for h in (0, 1):
                    rb_ps = ps.tile([64, CH], F32, tag="m", bufs=2, name=f"rb_{h}_{pc}")
                    nc.tensor.matmul(
                        rb_ps,
                        ones_sb[64:65, 0:64],
                        at2[64:65, h, :],
                        start=True,
                        stop=True,
                    )
                    nc.vector.tensor_mul(
                        attn_sb[64 * h : 64 * h + 64, tcol], at2[0:64, h, :], rb_ps
                    )

            store_engs = [nc.sync, nc.gpsimd]

            def emit_proj_tile(pc, m, scalar_evac=False):
                tcol = slice(CH * pc, CH * pc + CH)
                pr_ps = ps.tile([128, CH], F32, tag="m", bufs=2, name=f"pr_{m}_{pc}")
                nc.tensor.matmul(
                    pr_ps,
                    projT_sb[:, 128 * m : 128 * m + 128],
                    attn_sb[:, tcol],
                    start=True,
                    stop=True,
                )
                ob = outev_pool.tile([128, CH], BF16, tag="outev", name=f"ob_{m}_{pc}")
                if scalar_evac:
                    nc.scalar.copy(ob, pr_ps)
                else:
                    nc.vector.tensor_copy(ob, pr_ps)
                store_engs[m % 2].dma_start(out=out[pc, m], in_=ob)

            # q/k sweep-1 sliced into 1-bank [128, CH] sub-sweeps (tag "m")
            # so they interleave with D's scores without touching the s2
            # score-pair buffers.
            def qk_sub(part, c):
                g = ps.tile([128, CH], F32, tag="m", bufs=2, name=f"qks_{part}_{c}")
                cols = slice(128 * part, 128 * part + 128)
                for ct in range(NCT):
                    nc.tensor.matmul(
                        g,
                        w_sb[:, ct, cols],
                        x_sb[:, ct, CH * c : CH * c + CH],
                        start=(ct == 0),
                        stop=(ct == NCT - 1),
                    )
                dest = qT_sb if part == 0 else kT_sb
                nc.vector.tensor_scalar_add(
                    dest[:, CH * c : CH * c + CH], g, bias_sb[:, part : part + 1]
                )

            # fillers[(c, P)] -> list of emit thunks: stage-B leftovers spread
            # through the early chunks' pair loops as PE filler, each slice
            # timed to land before its first consumer.
            fillers = {
                (0, 0): [lambda: emit_v_chunk(1)],
                (0, 1): [lambda: transposes_for(1), lambda: qk_sub(0, 2)],
                (1, 0): [lambda: qk_sub(1, 2)],
                (1, 1): [lambda: emit_v_chunk(2)],
                (1, 2): [lambda: transposes_for(2)],
                (1, 3): [lambda: qk_sub(0, 3)],
                (2, 0): [lambda: qk_sub(1, 3)],
                (2, 1): [lambda: emit_v_chunk(3)],
                (2, 2): [lambda: transposes_for(3)],
            }

            # Chunk order (0, 1, 2, 3): the first chunks' slack absorbs the
            # stage-B leftovers, and the final chunk is the ScalarE-heaviest
            # (8 exp pairs), giving the PE the deepest well of parallel work
            # while only ONE chunk's norm+proj+store tail remains at the end.
            for c in (0, 1, 2, 3):
                npair = 2 * c + 2
                pv_ps = {
                    h: ps.tile([65, CH], F32, tag="pv", bufs=2, name=f"pv_{h}_{c}")
                    for h in (0, 1)
                }
                pending = []
                proj_emitted = 0

                def emit_pv(step):
                    for (pj, ph, pw, plane, plo) in step:
                        nc.tensor.matmul(
                            pv_ps[ph][:, plo:CH],
                            (v_aug0 if ph == 0 else v_aug1)[:, pj, :],
                            pw[:, plane, plo:CH],
                            start=(pj == 0),
                            stop=(pj == 4 * c + 3),
                        )

                for P in range(npair):
                    for h in (0, 1):
                        hrow = slice(64 * h, 64 * h + 64)
                        s2 = ps.tile(
                            [128, 2, CH], F32, tag="s2", bufs=2, name=f"s_{h}_{c}_{P}"
                        )
                        # pair-level column offset: both planes computed from
                        # slo_pair so the pair exp reads only written PSUM
                        # (the second diagonal plane recomputes 128 masked
                        # cols - 128 cycles, cheaper than a separate ACT)
                        slo_pair = max(0, 128 * (2 * P - 4 * c))
                        for i in (0, 1):
                            j = 2 * P + i
                            nc.tensor.matmul(
                                s2[:, i, slo_pair:CH],
                                kT_sb[hrow, 128 * j : 128 * j + 128],
                                qT_sb[hrow, CH * c + slo_pair : CH * c + CH],
                                start=True,
                                stop=True,
                            )
                        w2 = expw_pool.tile(
                            [128, 2, CH], BF16, tag="expw", name=f"w_{h}_{c}_{P}"
                        )
                        nc.scalar.activation(
                            out=w2[:, :, slo_pair:CH],
                            in_=s2[:, :, slo_pair:CH],
                            func=EXP,
                        )
                        for i in (0, 1):
                            j = 2 * P + i
                            diag = j - 4 * c
                            if diag >= 0:
                                # keep exp(score) where t >= k: within the
                                # kept column range f' = f - 128*diag, so
                                # f' - p >= 0
                                lo = 128 * diag if diag > 0 else 0
                                nc.gpsimd.affine_select(
                                    out=w2[:, i, lo:CH],
                                    in_=w2[:, i, lo:CH],
                                    pattern=[[1, CH - lo]],
                                    compare_op=IS_GE,
                                    fill=0.0,
                                    base=0,
                                    channel_multiplier=-1,
                                )
                            pending.append((j, h, w2, i, max(0, 128 * diag)))
                    if P == 0 and pending_fin is not None:
                        pv_prev, pc = pending_fin
                        pending_norm = (emit_fin(pv_prev, pc), pc)
                        pending_fin = None
                    if P == 2 and pending_norm is not None:
                        emit_norm(*pending_norm)
                        pending_norm = None
                    for thunk in fillers.get((c, P), ()):
                        thunk()
                    while len(pending) > 2 * PIPE:
                        step, pending = pending[:2], pending[2:]
                        emit_pv(step)
                    if pending_proj is not None and P >= 3:
                        target = ((P - 2) * 8) // max(npair - 3, 1)
                        while proj_emitted < min(target, 8):
                            emit_proj_tile(pending_proj, proj_emitted)
                            proj_emitted += 1
                while pending:
                    step, pending = pending[:2], pending[2:]
                    emit_pv(step)
                if pending_proj is not None:
                    while proj_emitted < 8:
                        emit_proj_tile(pending_proj, proj_emitted)
                        proj_emitted += 1

                pending_fin = (pv_ps, c)
                pending_proj = c

            # tail: the last chunk's finalize + norm + projections; ScalarE is
            # done with exps - let it evacuate half the proj tiles in parallel
            # with the DVE
            pv_prev, pc = pending_fin
            emit_norm(emit_fin(pv_prev, pc), pc)
            for m in range(8):
                emit_proj_tile(pending_proj, m, scalar_evac=(m % 2 == 1))

    nc.compile()
    return nc


def _get_nc():
    if "nc" not in _CACHE:
        _CACHE["nc"] = _build()
    return _CACHE["nc"]


def _make_in_maps(x, wqkv_w, wqkv_b, proj_w):
    bf = ml_dtypes.bfloat16
    xT = np.ascontiguousarray(np.asarray(x, np.float32).T.astype(bf))
    identb = np.eye(128, dtype=bf)
    ones_f = np.ones((128, 128), np.float32)
    scale = np.float32(1.0 / np.sqrt(C))
    in_maps = []
    for i in range(N_CORES):
        rows = []
        biases = []
        for blk, s in ((0, scale), (1, None), (2, None)):
            sl = slice(blk * C + 128 * i, blk * C + 128 * i + 128)
            w = np.asarray(wqkv_w[sl], np.float32)
            b = np.asarray(wqkv_b[sl], np.float32)
            if s is not None:
                w = w * s
                b = b * s
            rows.append(w)
            biases.append(b)
        W = np.concatenate(rows, axis=0)  # [384, 1024]
        B = np.stack(biases, axis=1)  # [128, 3]
        pT = np.asarray(proj_w[:, 128 * i : 128 * i + 128], np.float32).T  # [128, 1024]
        in_maps.append(
            {
                "xT": xT,
                "wqkv": np.ascontiguousarray(W.T.astype(bf)),
                "projT": np.ascontiguousarray(pT.astype(bf)),
                "identb": identb,
                "ones_f": ones_f,
                "bias": np.ascontiguousarray(B),
            }
        )
    return in_maps


def kernel(x, wqkv_w, wqkv_b, proj_w, proj_b, _trace=False, _tmpdir=None):
    from concourse.bass_utils import run_bass_kernel_spmd

    nc = _get_nc()
    in_maps = _make_in_maps(x, wqkv_w, wqkv_b, proj_w)
    res = run_bass_kernel_spmd(
        nc,
        in_maps,
        core_ids=list(range(N_CORES)),
        trace=_trace,
        tmpdir=_tmpdir,
    )
    acc = np.zeros((NT, 8, 128, CH), np.float64)
    for rmap in res.results:
        acc += rmap["out"].astype(np.float64)
    partialT = acc.transpose(1, 2, 0, 3).reshape(C, T)  # [o, t]
    full = partialT.T + np.asarray(proj_b, np.float64)[None, :]
    if _trace:
        _CACHE["last_result"] = res
    return full.astype(np.float32)
